# revision 57
# baseline (speedup 1.0000x reference)
"""Qwen-style GQA full attention (B=2, S=2048, HID=2048, H=16, KVH=8, D=128)
on 8 trn2 NeuronCores.

Sharding: tensor-parallel across head groups. Core d owns kv-head d and its
two query heads (2d, 2d+1): Wq/Wk/Wv column shards, Wo row shard. Each core
computes a partial [B*S, HID] output (its 2 heads' contribution through its
Wo row block); the host sums the 8 partials.

Device kernel (per core):
  phase 1  QKV+gate projection in fp8e4m3 DoubleRow matmuls (K=256 per
           instruction, 0.5 cyc/row). hs and W are shipped as (hi, lo)
           e4m3 residual pairs at one power-of-2 scale; q/k/v use the
           3-cross-term product (~bf16 accuracy at 0.75x bf16 PE cost),
           gates use the single hi*hi term (the sigmoid damps the noise;
           device-measured total rel err 1.59e-2 < 2e-2). Per-head RMSNorm
           via Pool
           partition_all_reduce of x^2; the fp8 scale is unfolded inside
           exp(-0.5*ln(ss*scale+eps) - ln(s)). RoPE via half-rotated
           sin/cos tables (norm weight + 1/sqrt(D) folded in host-side).
  phase 2  V transposed to token-major via PE transposes (fused per chunk).
  phase 3  causal attention with batch-0/batch-1 pairs fused into one
           [128,1024] stream (identical causal extents): scoresT = K-chunk
           stationary x moving Q -> one exp -> fused diagonal-block mask
           -> per-half PV and ones128 broadcast row-sums, software-
           pipelined 2 steps deep ACROSS pair boundaries so the PE never
           drains. Raw pv + sum rows are stashed to SBUF.
  phase 4  deferred per-pair normalize+gate (1/sum = exp(-ln(sum)) on Act,
           rank-1 broadcast matmul, the V/gate fp8 scales cancel inside
           gtb = sigmoid/SXW), then the Wo row-shard projection into
           [128,1024] PSUM accumulators (4 deep); psum->sbuf copies
           alternate Act/DVE (GPSIMD cannot read PSUM), DMA out in bf16
           with the final tile split across both HWDGE queues.
"""

import os
import numpy as np
import ml_dtypes

import concourse.bass as bass
import concourse.tile as tile
from concourse import bacc, mybir
from concourse.bass_isa import ReduceOp
from contextlib import ExitStack

BF16 = ml_dtypes.bfloat16
E4M3 = ml_dtypes.float8_e4m3
F32 = mybir.dt.float32
BF = mybir.dt.bfloat16
F8 = mybir.dt.float8e4
AF = mybir.ActivationFunctionType
DR = mybir.MatmulPerfMode.DoubleRow

class _Bacc(bacc.Bacc):
    """Bacc that prefers the combined Ln+Exp activation table set, so the
    kernel's Ln/Exp/Copy mix resolves to a single ACT_TABLE_LOAD instead of
    thrashing between exp_and_others and natural_log (~2.7us per switch)."""

    def insert_act_table_loads(self):
        import bass_rust as _bass_rust
        from concourse.hw_specs import get_activation_tables
        has_activation = any(
            isinstance(i, mybir.InstActivation)
            for b in self.main_func.blocks
            for i in b.instructions
        )
        if not has_activation:
            return
        items = [
            (nm, fns if nm == "natural_log_exp_and_others" else set())
            for nm, fns in get_activation_tables(self.m.arch).items()
        ]
        _bass_rust.insert_act_table_loads(self, items)


B, S, HID, H, KVH, D = 2, 2048, 2048, 16, 8, 128
G = H // KVH              # q heads per kv head (= per core)
EPS = 1e-6
SCALE = D ** -0.5
CH = 512                  # token chunk (proj phase)
NCORES = 8

SX = 2.0 ** 5             # fp8 scale on hidden states
SW = 2.0 ** 10            # fp8 scale on Wq/Wk/Wv
SXW = SX * SW             # scale of every projection psum
LN_SXW = float(np.log(SXW))


def build_nc(S_=S):
    """Build the single-core SPMD program (identical on all 8 cores)."""
    HC = HID // 128           # hid chunks
    HP = HC // 2              # DoubleRow K-256 pairs
    N = B * S_                # total tokens
    SK = S_ // 128            # k-tiles per batch
    NP = S_ // 256            # q-tile pairs per batch
    CPB = S_ // CH            # token chunks per batch
    NT = CH // 128            # 128-tok tiles per chunk

    nc = _Bacc(None)
    nc._phase_marks = []
    _mark = lambda s: nc._phase_marks.append((s, nc.next_id()))

    hsh_d = nc.dram_tensor("hsh", [HID, N], F8, kind="ExternalInput")
    hsl_d = nc.dram_tensor("hsl", [HID, N], F8, kind="ExternalInput")
    wqh_d = nc.dram_tensor("wqh", [HC, 128, 512], F8, kind="ExternalInput")
    wql_d = nc.dram_tensor("wql", [HC, 128, 512], F8, kind="ExternalInput")
    wkh_d = nc.dram_tensor("wkh", [HC, 128, 128], F8, kind="ExternalInput")
    wkl_d = nc.dram_tensor("wkl", [HC, 128, 128], F8, kind="ExternalInput")
    wvh_d = nc.dram_tensor("wvh", [HC, 128, 128], F8, kind="ExternalInput")
    wvl_d = nc.dram_tensor("wvl", [HC, 128, 128], F8, kind="ExternalInput")
    wo_d = nc.dram_tensor("wo", [G, 128, HID], BF, kind="ExternalInput")
    cq_d = nc.dram_tensor("cosq", [128, S_], BF, kind="ExternalInput")
    sq_d = nc.dram_tensor("sinq", [128, S_], BF, kind="ExternalInput")
    ck_d = nc.dram_tensor("cosk", [128, S_], BF, kind="ExternalInput")
    sk_d = nc.dram_tensor("sink", [128, S_], BF, kind="ExternalInput")
    id_d = nc.dram_tensor("ident", [128, 128], BF, kind="ExternalInput")
    o128_d = nc.dram_tensor("ones128", [128, 128], BF, kind="ExternalInput")
    ma_d = nc.dram_tensor("maska", [128, 1024], BF, kind="ExternalInput")
    mb_d = nc.dram_tensor("maskb", [128, 1024], BF, kind="ExternalInput")
    out_d = nc.dram_tensor("out", [N, HID], BF, kind="ExternalOutput")

    with tile.TileContext(nc) as tc, ExitStack() as ctx:
        cpool = ctx.enter_context(tc.tile_pool(name="consts", bufs=1))

        wqh_s = cpool.tile([128, HC, 512], F8)
        wql_s = cpool.tile([128, HC, 512], F8)
        wkh_s = cpool.tile([128, HC, 128], F8)
        wkl_s = cpool.tile([128, HC, 128], F8)
        wvh_s = cpool.tile([128, HC, 128], F8)
        wvl_s = cpool.tile([128, HC, 128], F8)
        wo_s = cpool.tile([128, G, HID], BF)
        cq_s = cpool.tile([128, S_], BF)
        sq_s = cpool.tile([128, S_], BF)
        ck_s = cpool.tile([128, S_], BF)
        sk_s = cpool.tile([128, S_], BF)
        id_s = cpool.tile([128, 128], BF)
        o128_s = cpool.tile([128, 128], BF)
        ma_s = cpool.tile([128, 1024], BF)
        mb_s = cpool.tile([128, 1024], BF)
        epsb = cpool.tile([128, 1], F32)
        oneb = cpool.tile([128, 1], F32)
        lnsb = cpool.tile([128, 1], F32)
        nc.vector.memset(epsb[:], EPS)
        nc.vector.memset(oneb[:], 1.0)
        nc.vector.memset(lnsb[:], -LN_SXW)

        _mark('consts')
        # persistent activations (feature-major: [D, ...tok])
        qtb = cpool.tile([128, B, SK, G, 128], BF)   # rope'd+normed q
        ktb = cpool.tile([128, B, SK, 128], BF)      # rope'd+normed k
        vtb = cpool.tile([128, N], BF)               # v*SXW, feature-major
        vb = cpool.tile([128, B, SK, 128], BF)       # v*SXW, token-major
        gtb = cpool.tile([128, NP, B, 2, G, 128], BF)  # sigmoid(gate)/SXW
        pvs = cpool.tile([128, NP, 1024], BF)        # raw pv stash
        sms = cpool.tile([1, NP, 1024], BF)          # softmax sum rows

        hsh_v = hsh_d[:].rearrange("(c p) n -> c p n", p=128)
        hsl_v = hsl_d[:].rearrange("(c p) n -> c p n", p=128)

        # ---------------- phase 1: projections ----------------
        with (
            tc.tile_pool(name="hst", bufs=2) as hstp,
            tc.tile_pool(name="projps", bufs=6, space="PSUM") as projps,
            tc.tile_pool(name="pwork", bufs=2) as pwork,
        ):
            # pre-allocate hs chunk tiles so the DMA schedule below can
            # reference them; the pool still rotates 2 bufs per tag (WAR
            # deps on the queue serialize refills correctly)
            ht_tiles = {}
            for b in range(B):
                for cc in range(CPB):
                    ht_tiles[b, cc] = (
                        hstp.tile([128, HC, CH], F8, tag="hsth",
                                  name=f"hth{b}{cc}"),
                        hstp.tile([128, HC, CH], F8, tag="hstl",
                                  name=f"htl{b}{cc}"),
                    )

            def dma_ht(b, cc):
                t0 = b * S_ + cc * CH
                hth, htl = ht_tiles[b, cc]
                nc.sync.dma_start(
                    hth[:], hsh_v[:, :, t0:t0 + CH].rearrange("c p f -> p c f"))
                nc.scalar.dma_start(
                    htl[:], hsl_v[:, :, t0:t0 + CH].rearrange("c p f -> p c f"))

            # DMA schedule: one big DMA per tensor (per-DMA queue overhead
            # ~0.6us dominates small transfers). Need-ordered per queue.
            # First chunk + wqh interleaved in halves for an early start.
            h8 = HC // 2
            hth0, htl0 = ht_tiles[0, 0]
            nc.sync.dma_start(wqh_s[:, 0:h8, :],
                              wqh_d[0:h8].rearrange("c p f -> p c f"))
            nc.sync.dma_start(
                hth0[:, 0:h8, :],
                hsh_v[0:h8, :, 0:CH].rearrange("c p f -> p c f"))
            nc.scalar.dma_start(
                htl0[:, 0:h8, :],
                hsl_v[0:h8, :, 0:CH].rearrange("c p f -> p c f"))
            nc.sync.dma_start(wqh_s[:, h8:HC, :],
                              wqh_d[h8:HC].rearrange("c p f -> p c f"))
            nc.sync.dma_start(
                hth0[:, h8:HC, :],
                hsh_v[h8:HC, :, 0:CH].rearrange("c p f -> p c f"))
            nc.scalar.dma_start(
                htl0[:, h8:HC, :],
                hsl_v[h8:HC, :, 0:CH].rearrange("c p f -> p c f"))
            nc.scalar.dma_start(wql_s[:], wql_d[:].rearrange("c p f -> p c f"))
            nc.sync.dma_start(wkh_s[:], wkh_d[:].rearrange("c p f -> p c f"))
            hth01, htl01 = ht_tiles[0, 1]
            nc.sync.dma_start(
                hth01[:], hsh_v[:, :, CH:2 * CH].rearrange("c p f -> p c f"))
            nc.scalar.dma_start(cq_s[:], cq_d[:])
            nc.scalar.dma_start(sq_s[:], sq_d[:])
            nc.sync.dma_start(wkl_s[:], wkl_d[:].rearrange("c p f -> p c f"))
            nc.sync.dma_start(wvh_s[:], wvh_d[:].rearrange("c p f -> p c f"))
            nc.sync.dma_start(wvl_s[:], wvl_d[:].rearrange("c p f -> p c f"))
            nc.scalar.dma_start(ck_s[:], ck_d[:])
            nc.scalar.dma_start(sk_s[:], sk_d[:])
            nc.scalar.dma_start(
                htl01[:], hsl_v[:, :, CH:2 * CH].rearrange("c p f -> p c f"))
            nc.scalar.dma_start(ma_s[:], ma_d[:])
            nc.scalar.dma_start(mb_s[:], mb_d[:])
            nc.scalar.dma_start(id_s[:], id_d[:])
            nc.scalar.dma_start(o128_s[:], o128_d[:])
            dma_ht(0, 2)
            dma_ht(0, 3)
            for cc in range(CPB):
                dma_ht(1, cc)
            nc.scalar.dma_start(wo_s[:], wo_d[:].rearrange("c p f -> p c f"))

            for b in range(B):
                for cc in range(CPB):
                    t0 = b * S_ + cc * CH     # global token start
                    p0 = cc * CH              # position start (within batch)
                    hth, htl = ht_tiles[b, cc]

                    psq0 = projps.tile([128, CH], F32, tag="pp", name="psq0")
                    psq1 = projps.tile([128, CH], F32, tag="pp", name="psq1")
                    psk = projps.tile([128, CH], F32, tag="pp", name="psk")
                    psv = projps.tile([128, CH], F32, tag="pp", name="psv")
                    psg0 = projps.tile([128, CH], F32, tag="pp", name="psg0")
                    psg1 = projps.tile([128, CH], F32, tag="pp", name="psg1")

                    # 3-term residual fp8: Wh@xh + Wh@xl + Wl@xh
                    # (wl last: its DMA trails wh on the queue)
                    def dr3(ps, wh, wl, lo, hi):
                        n3 = 3 * HP
                        i = 0
                        for wt, xt in ((wh, hth), (wh, htl), (wl, hth)):
                            for c in range(HP):
                                nc.tensor.matmul(
                                    ps, wt[:, 2 * c:2 * c + 2, lo:hi],
                                    xt[:, 2 * c:2 * c + 2, :],
                                    start=(i == 0), stop=(i == n3 - 1),
                                    perf_mode=DR)
                                i += 1

                    # 1-term fp8 (gates): Wh@xh — the sigmoid damps
                    # quantization noise; total rel err stays under the gate
                    def dr2(ps, wh, wl, lo, hi):
                        for c in range(HP):
                            nc.tensor.matmul(
                                ps, wh[:, 2 * c:2 * c + 2, lo:hi],
                                hth[:, 2 * c:2 * c + 2, :],
                                start=(c == 0), stop=(c == HP - 1),
                                perf_mode=DR)

                    dr3(psq0[:], wqh_s, wql_s, 0, 128)
                    dr3(psq1[:], wqh_s, wql_s, 128, 256)
                    dr3(psk[:], wkh_s, wkl_s, 0, 128)
                    dr3(psv[:], wvh_s, wvl_s, 0, 128)
                    dr2(psg0[:], wqh_s, wql_s, 256, 384)
                    dr2(psg1[:], wqh_s, wql_s, 384, 512)

                    ti0 = cc * NT
                    # RMSNorm + RoPE for q heads and k (inputs scaled by SXW;
                    # the unscale is folded into the Exp bias: rstd/SXW)
                    blocks = [
                        (psq0, cq_s, sq_s, qtb[:, b, ti0:ti0 + NT, 0, :]),
                        (psq1, cq_s, sq_s, qtb[:, b, ti0:ti0 + NT, 1, :]),
                        (psk, ck_s, sk_s, ktb[:, b, ti0:ti0 + NT, :]),
                    ]
                    for psx, ctab, stab, dest in blocks:
                        xu = pwork.tile([128, CH], BF, tag="xu", name="xu")
                        nc.scalar.copy(xu[:], psx[:])
                        xsq = pwork.tile([128, CH], BF, tag="xsq", name="xsq")
                        nc.vector.tensor_mul(xsq[:], xu[:], xu[:])
                        ssb = pwork.tile([128, CH], F32, tag="ssb", name="ssb")
                        nc.gpsimd.partition_all_reduce(ssb[:], xsq[:], 128,
                                                       ReduceOp.add)
                        ssl = pwork.tile([128, CH], F32, tag="ssl", name="ssl")
                        nc.scalar.activation(ssl[:], ssb[:], AF.Ln,
                                             bias=epsb[:],
                                             scale=1.0 / (D * SXW * SXW))
                        rsts = pwork.tile([128, CH], BF, tag="rsts",
                                          name="rsts")
                        nc.scalar.activation(rsts[:], ssl[:], AF.Exp,
                                             scale=-0.5, bias=lnsb[:])
                        t1 = pwork.tile([128, CH], BF, tag="t1", name="t1")
                        nc.vector.tensor_mul(t1[:], xu[:], ctab[:, p0:p0 + CH])
                        xrot = pwork.tile([128, CH], BF, tag="xrot",
                                          name="xrot")
                        nc.vector.tensor_copy(xrot[0:64, :], xu[64:128, :])
                        nc.vector.tensor_copy(xrot[64:128, :], xu[0:64, :])
                        t2 = pwork.tile([128, CH], BF, tag="t2", name="t2")
                        nc.vector.tensor_mul(t2[:], xrot[:],
                                             stab[:, p0:p0 + CH])
                        nc.vector.tensor_add(t1[:], t1[:], t2[:])
                        nc.vector.tensor_mul(dest, t1[:], rsts[:])

                    # v: stash feature-major, still scaled by SXW (the
                    # unscale rides in gtb = sigmoid/SXW)
                    nc.scalar.copy(vtb[:, t0:t0 + CH], psv[:])

                    # gates: sigmoid(g)/SXW = exp(-ln(1 + exp(-g)) - ln SXW)
                    for hh, psg in ((0, psg0), (1, psg1)):
                        e1 = pwork.tile([128, CH], BF, tag="e1", name="e1")
                        nc.scalar.activation(e1[:], psg[:], AF.Exp,
                                             scale=-1.0 / SXW)
                        l1 = pwork.tile([128, CH], F32, tag="l1", name="l1")
                        nc.scalar.activation(l1[:], e1[:], AF.Ln, bias=oneb[:])
                        nc.scalar.activation(
                            gtb[:, 2 * cc:2 * cc + 2, b, :, hh, :],
                            l1[:], AF.Exp, scale=-1.0, bias=lnsb[:])

                    # phase 2 fused in: this chunk's V -> token-major
                    vt_ps = projps.tile([128, 512], BF, tag="pp",
                                        name="vt_ps")
                    for jj in range(NT):
                        nc.tensor.transpose(
                            vt_ps[:, jj * 128:(jj + 1) * 128],
                            vtb[:, t0 + jj * 128: t0 + (jj + 1) * 128],
                            id_s[:])
                    nc.vector.tensor_copy(vb[:, b, ti0:ti0 + NT, :], vt_ps[:])
                    _mark(f'proj b{b}c{cc}')

        # ---------------- phase 3: attention (batches fused) ----------------
        with (
            tc.tile_pool(name="scps", bufs=2, space="PSUM") as scps,
            tc.tile_pool(name="pvps", bufs=1, space="PSUM") as pvps,
            tc.tile_pool(name="sumps", bufs=1, space="PSUM") as sumps,
            tc.tile_pool(name="probsp", bufs=4) as probsp,
        ):
            # One flat software-pipelined stream over all (pair, j) steps:
            # PV/sum trail QK/exp by 2 steps and flow ACROSS pair
            # boundaries, so the PE never drains. Per-pair state (pv, sums)
            # is carried in the pend queue entries.
            def gate_tail(p, pv, smAB):
                # stash raw pv + the sum row; normalization and gating are
                # deferred to phase 4 where Act/DVE have headroom (keeping
                # the attention Act budget = exps only, below the PE)
                nc.vector.tensor_copy(pvs[:, p, :], pv[:])
                nc.vector.tensor_copy(sms[:, p, :], smAB[0:1, :])

            steps = [(p, j) for p in range(NP) for j in range(2 * p + 2)]
            pend = []
            state = {}

            def pop_pend():
                p, jp, pprobs = pend.pop(0)
                pv, smAB = state[p]
                st, sp = jp == 0, jp == 2 * p + 1
                nc.tensor.matmul(pv[:, 0:512], vb[:, 0, jp, :],
                                 pprobs[:, 0:512], start=st, stop=sp)
                nc.tensor.matmul(pv[:, 512:1024], vb[:, 1, jp, :],
                                 pprobs[:, 512:1024], start=st, stop=sp)
                nc.tensor.matmul(smAB[:, 0:512], o128_s[:],
                                 pprobs[:, 0:512], start=st, stop=sp)
                nc.tensor.matmul(smAB[:, 512:1024], o128_s[:],
                                 pprobs[:, 512:1024], start=st, stop=sp)
                if sp:
                    gate_tail(p, pv, smAB)
                    del state[p]
                    _mark(f'attn p{p}')

            for p, j in steps:
                i0, jmax = 2 * p, 2 * p + 1
                if j == 0:
                    state[p] = (pvps.tile([128, 1024], F32, tag="pv",
                                          name="pv"),
                                sumps.tile([128, 1024], F32, tag="sm",
                                           name="smAB"))
                scp = scps.tile([128, 1024], F32, tag="sc", name="scp")
                nc.tensor.matmul(scp[:, 0:512], ktb[:, 0, j, :],
                                 qtb[:, 0, i0:i0 + 2, :, :])
                nc.tensor.matmul(scp[:, 512:1024], ktb[:, 1, j, :],
                                 qtb[:, 1, i0:i0 + 2, :, :])
                probs = probsp.tile([128, 1024], BF, tag="probs",
                                    name="probs")
                nc.scalar.activation(probs[:], scp[:], AF.Exp)
                if j == i0:
                    nc.vector.tensor_mul(probs[:], probs[:], ma_s[:])
                elif j == jmax:
                    nc.vector.tensor_mul(probs[:], probs[:], mb_s[:])
                pend.append((p, j, probs))
                if len(pend) > 2:
                    pop_pend()
            while pend:
                pop_pend()

        # ---------------- phase 4: normalize+gate, then Wo ----------------
        with (
            tc.tile_pool(name="wops", bufs=3, space="PSUM") as wops,
            tc.tile_pool(name="rsps", bufs=1, space="PSUM") as rsps,
            tc.tile_pool(name="osbp", bufs=3) as osbp,
            tc.tile_pool(name="wwork", bufs=3) as wwork,
            tc.tile_pool(name="gallp", bufs=3) as gallp,
        ):
            # GPSIMD cannot read PSUM, so Act and DVE alternate on the
            # psum->sbuf copies; [128,1024] half-tile accumulators with 3
            # in flight keep the copies off the PE critical path.
            cpeng = [lambda o, i: nc.scalar.copy(o, i),
                     lambda o, i: nc.vector.tensor_copy(o, i)]
            dmeng = [nc.sync, nc.scalar]
            ti = 0
            galls = {}

            def pair_chain(p):
                # deferred softmax rescale + gate: gall = pv*(1/sum)*gate
                lsb = wwork.tile([1, 1024], F32, tag="lsb", name="lsb")
                nc.scalar.activation(lsb[:], sms[:, p, :], AF.Ln)
                rsb = wwork.tile([1, 1024], BF, tag="rsb", name="rsb")
                nc.scalar.activation(rsb[:], lsb[:], AF.Exp, scale=-1.0)
                rsbB = rsps.tile([128, 1024], F32, tag="rs", name="rsbB")
                nc.tensor.matmul(rsbB[:, 0:512], o128_s[0:1, :],
                                 rsb[:, 0:512])
                nc.tensor.matmul(rsbB[:, 512:1024], o128_s[0:1, :],
                                 rsb[:, 512:1024])
                tmp = wwork.tile([128, 1024], BF, tag="tmp", name="tmp")
                nc.vector.tensor_mul(tmp[:], pvs[:, p, :], rsbB[:])
                gall = gallp.tile([128, 1024], BF, tag="gall", name="gall")
                nc.vector.tensor_mul(gall[:], tmp[:], gtb[:, p, :])
                galls[p] = gall

            def pair_tiles(p):
                nonlocal ti
                gall = galls.pop(p)
                lastp = p == NP - 1
                for b in range(B):
                    for it in range(2):
                        trow = b * S_ + (2 * p + it) * 128
                        g0 = gall[:, b * 512 + it * 256:
                                  b * 512 + it * 256 + 128]
                        g1 = gall[:, b * 512 + it * 256 + 128:
                                  b * 512 + it * 256 + 256]
                        osb = osbp.tile([128, HID], BF, tag="osb", name="osb")
                        last = lastp and b == B - 1 and it == 1
                        for half in range(2):
                            wop = wops.tile([128, 1024], F32, tag="wo",
                                            name="wop")
                            for o2 in range(2):
                                oc = half * 2 + o2
                                nc.tensor.matmul(
                                    wop[:, o2 * 512:(o2 + 1) * 512], g0,
                                    wo_s[:, 0, oc * 512:(oc + 1) * 512],
                                    start=True, stop=False)
                                nc.tensor.matmul(
                                    wop[:, o2 * 512:(o2 + 1) * 512], g1,
                                    wo_s[:, 1, oc * 512:(oc + 1) * 512],
                                    start=False, stop=True)
                            cpeng[ti % 2](
                                osb[:, half * 1024:(half + 1) * 1024],
                                wop[:])
                            ti += 1
                            # drain the final tile with per-half DMAs on
                            # both queues so the tail isn't serialized
                            if last:
                                dmeng[half].dma_start(
                                    out_d[trow:trow + 128,
                                          half * 1024:(half + 1) * 1024],
                                    osb[:, half * 1024:(half + 1) * 1024])
                        if not last:
                            dmeng[ti % 2].dma_start(
                                out_d[trow:trow + 128, :], osb[:])

            for p in range(NP + 1):
                if p < NP:
                    pair_chain(p)
                if p >= 1:
                    pair_tiles(p - 1)
            _mark('wo')
    nc.compile()
    return nc


def prep_inputs(hidden_states, cos, sin, Wq, Wk, Wv, Wo, q_norm_w, k_norm_w,
                S_=S):
    """Host-side sharding + layout prep. Returns in_maps for 8 cores."""
    N = B * S_
    hsT = np.ascontiguousarray(
        np.asarray(hidden_states, np.float32).reshape(N, HID).T)

    def split8(x, s):
        hi = np.asarray(x * s, E4M3)
        lo = np.asarray(x * s - hi.astype(np.float32), E4M3)
        return hi, lo

    hsh, hsl = split8(hsT, SX)

    cos0 = np.asarray(cos[0], np.float32)   # [S_, D] (identical across batch)
    sin0 = np.asarray(sin[0], np.float32)
    qw = np.asarray(q_norm_w, np.float32)
    kw = np.asarray(k_norm_w, np.float32)
    sign = np.where(np.arange(D) < 64, -1.0, 1.0).astype(np.float32)
    shift = (np.arange(D) + 64) % D

    cosq = np.ascontiguousarray(cos0.T * qw[:, None] * SCALE).astype(BF16)
    sinq = np.ascontiguousarray(
        sin0.T * (sign * qw[shift])[:, None] * SCALE).astype(BF16)
    cosk = np.ascontiguousarray(cos0.T * kw[:, None]).astype(BF16)
    sink = np.ascontiguousarray(
        sin0.T * (sign * kw[shift])[:, None]).astype(BF16)

    tri = (np.arange(128)[:, None] <= np.arange(128)[None, :])
    onesq = np.ones((128, 128), np.float32)
    ma5 = np.concatenate([tri, tri, onesq, onesq], axis=1)
    mb5 = np.concatenate([0 * onesq, 0 * onesq, tri, tri], axis=1)
    maska = np.concatenate([ma5, ma5], axis=1).astype(BF16)
    maskb = np.concatenate([mb5, mb5], axis=1).astype(BF16)
    ident = np.eye(128, dtype=BF16)
    ones128 = np.ones((128, 128), BF16)

    HC = HID // 128
    Wq = np.asarray(Wq, np.float32)
    Wk = np.asarray(Wk, np.float32)
    Wv = np.asarray(Wv, np.float32)
    Wo = np.asarray(Wo, np.float32)
    in_maps = []
    for d in range(NCORES):
        h0, h1 = G * d, G * d + 1
        q0 = Wq[:, h0 * 2 * D: h0 * 2 * D + D]
        g0 = Wq[:, h0 * 2 * D + D: (h0 + 1) * 2 * D]
        q1 = Wq[:, h1 * 2 * D: h1 * 2 * D + D]
        g1 = Wq[:, h1 * 2 * D + D: (h1 + 1) * 2 * D]
        wq_c = np.ascontiguousarray(
            np.concatenate([q0, q1, g0, g1], axis=1))       # [HID, 512]
        wqh, wql = split8(wq_c, SW)
        wkh, wkl = split8(
            np.ascontiguousarray(Wk[:, d * D:(d + 1) * D]), SW)
        wvh, wvl = split8(
            np.ascontiguousarray(Wv[:, d * D:(d + 1) * D]), SW)
        wo_a = np.ascontiguousarray(
            Wo[d * G * D:(d + 1) * G * D, :]).astype(BF16).reshape(G, 128, HID)
        in_maps.append({
            "hsh": hsh, "hsl": hsl,
            "wqh": wqh.reshape(HC, 128, 512),
            "wql": wql.reshape(HC, 128, 512),
            "wkh": wkh.reshape(HC, 128, 128),
            "wkl": wkl.reshape(HC, 128, 128),
            "wvh": wvh.reshape(HC, 128, 128),
            "wvl": wvl.reshape(HC, 128, 128),
            "wo": wo_a,
            "cosq": cosq, "sinq": sinq, "cosk": cosk, "sink": sink,
            "ident": ident, "ones128": ones128,
            "maska": maska, "maskb": maskb,
        })
    return in_maps


_NC_CACHE = {}
_RUNNER_CACHE = {}


def _get_nc(S_=S):
    if S_ not in _NC_CACHE:
        _NC_CACHE[S_] = build_nc(S_)
    return _NC_CACHE[S_]


def _get_runner(S_=S):
    """Build a cached jitted 8-core executable.

    Mirrors concourse.bass2jax.run_bass_via_pjrt's multi-core path, but
    keeps the jitted function (and device-resident output placeholders)
    so repeated calls don't re-trace/re-compile, and so the executable
    can be timed in a steady-state loop.
    """
    if S_ in _RUNNER_CACHE:
        return _RUNNER_CACHE[S_]
    import jax
    from jax.experimental.shard_map import shard_map
    from jax.sharding import Mesh, PartitionSpec
    from concourse import bass2jax, mybir as _mybir
    bass2jax.install_neuronx_cc_hook()

    nc = _get_nc(S_)
    assert nc.dbg_addr is None
    pid_name = (nc.partition_id_tensor.name
                if nc.partition_id_tensor is not None else None)

    in_names, out_names, out_avals = [], [], []
    for alloc in nc.m.functions[0].allocations:
        if not isinstance(alloc, _mybir.MemoryLocationSet):
            continue
        name = alloc.memorylocations[0].name
        if alloc.kind == "ExternalInput":
            if name != pid_name:
                in_names.append(name)
        elif alloc.kind == "ExternalOutput":
            out_names.append(name)
            out_avals.append(jax.core.ShapedArray(
                tuple(alloc.tensor_shape), _mybir.dt.np(alloc.dtype)))
    n_params = len(in_names)
    all_names = in_names + out_names
    if pid_name is not None:
        all_names = all_names + [pid_name]

    def _body(*args):
        operands = list(args)
        if pid_name is not None:
            operands.append(bass2jax.partition_id_tensor())
        outs = bass2jax._bass_exec_p.bind(
            *operands,
            out_avals=tuple(out_avals),
            in_names=tuple(all_names),
            out_names=tuple(out_names),
            lowering_input_output_aliases=(),
            sim_require_finite=True,
            sim_require_nnan=True,
            nc=nc,
        )
        return tuple(outs)

    devices = jax.devices()[:NCORES]
    mesh = Mesh(np.asarray(devices), ("core",))
    nin = n_params + len(out_names)
    sharded = jax.jit(
        shard_map(_body, mesh=mesh,
                  in_specs=(PartitionSpec("core"),) * nin,
                  out_specs=(PartitionSpec("core"),) * len(out_names),
                  check_rep=False),
        keep_unused=True,
    )
    zeros = [np.zeros((NCORES * a.shape[0], *a.shape[1:]), a.dtype)
             for a in out_avals]
    zeros_dev = [jax.device_put(z) for z in zeros]

    def run(in_maps):
        concat_in = [
            np.concatenate([np.asarray(m[nm]) for m in in_maps], axis=0)
            for nm in in_names
        ]
        outs = sharded(*concat_in, *zeros_dev)
        return {nm: np.asarray(outs[i]) for i, nm in enumerate(out_names)}

    def run_prepared(dev_args):
        return sharded(*dev_args, *zeros_dev)

    def prepare(in_maps):
        return [
            jax.device_put(np.concatenate(
                [np.asarray(m[nm]) for m in in_maps], axis=0))
            for nm in in_names
        ]

    r = {"run": run, "prepare": prepare, "run_prepared": run_prepared,
         "out_names": out_names, "out_avals": out_avals}
    _RUNNER_CACHE[S_] = r
    return r


def kernel(hidden_states, cos, sin, Wq, Wk, Wv, Wo, q_norm_w, k_norm_w):
    in_maps = prep_inputs(hidden_states, cos, sin, Wq, Wk, Wv, Wo,
                          q_norm_w, k_norm_w)
    runner = _get_runner()
    outs = runner["run"](in_maps)
    full = outs["out"].reshape(NCORES, B * S, HID)
    acc = full.astype(np.float32).sum(axis=0)
    return acc.reshape(B, S, HID)
